# revision 107
# baseline (speedup 1.0000x reference)
"""LocalMHA (windowed attention) Trainium2 Bass kernel.

Full inputs -> full outputs. Internally: 8-way data-parallel over
(batch, token-half) shards; each NeuronCore runs the complete block on
4096 tokens (32 windows of 128). No collectives.

Problem (hardcoded):
  x: (4, 1024, 8192) f32, DIM=1024, HEADS=16, DIM_HEAD=64, WINDOW=128
  out = W_out @ attn(LN(x)) + x   (per reference.py)
"""

import numpy as np
import ml_dtypes

import concourse.bass as bass
import concourse.bacc as bacc
import concourse.tile as tile
from concourse import mybir
from concourse.bass_utils import run_bass_kernel_spmd

BF16 = mybir.dt.bfloat16
F32 = mybir.dt.float32
FP8 = mybir.dt.float8e4
DR = mybir.MatmulPerfMode.DoubleRow

# fp8 scale factors: weights x SW_W, activations x SW_X; PSUM carries
# SW_W*SW_X, descaled in the PSUM->SBUF copy.
SW_W = 32.0
SW_X = 16.0
SW = SW_W * SW_X

B, DIM, T = 4, 1024, 8192
HEADS, DHEAD, WIN = 16, 64, 128
NCORE = 8
NTOK = (B * T) // NCORE          # 4096 tokens per core
NT = 512                         # token tile
NTILES = NTOK // NT              # 8
KC = DIM // 128                  # 8 contraction chunks
WPT = NT // WIN                  # 4 windows per token tile
ROPE_LAG = 2                     # chunks of lag before rope S-matmul
EPS = 1e-5

_CACHED = {}
PHASE_LOG = []

# build-time tuning knobs (swept via TimelineSim)
CFG = dict(
    sel_dma=True,       # denom broadcast via DRAM-bounce DMA vs sel matmuls
    sq_gpsimd=False,    # x^2 on gpsimd vs ACT
    psa_bufs=5,
    psb_bufs=1,
    have_bias=False,    # ln_b != 0: emit bias rank-1 in the vT path
    fp8_qk=False,       # q,k projection GEMM in fp8 DoubleRow
    fp8_vt=False,       # v projection GEMM in fp8 DoubleRow
    fp8_out=False,      # out projection GEMM in fp8 DoubleRow
)


def _mark(nc, phase):
    PHASE_LOG.append((phase, len(nc.inst_map)))


def _legalize_waits(nc):
    """This toolchain's walrus encodes at most ONE sem-wait per instruction
    (ISA EVENTS struct has a single wait slot) and errors with 'Too many sync
    wait commands' on Tile's multi-wait output. Split: hoist all but one wait
    onto same-engine ENGINE_NOPs inserted immediately before the instruction
    (engine stalls there first -> identical ordering semantics)."""
    eng_map = {
        mybir.EngineType.PE: nc.tensor,
        mybir.EngineType.Activation: nc.scalar,
        mybir.EngineType.DVE: nc.vector,
        mybir.EngineType.Pool: nc.gpsimd,
        mybir.EngineType.SP: nc.sync,
    }
    for f in nc.m.functions:
        for bb in f.blocks:
            lst = bb.instructions  # live list
            need = [
                i for i in lst
                if i.sync_info is not None and len(i.sync_info.on_wait) > 1
            ]
            for inst in need:
                si = inst.sync_info
                waits = list(si.on_wait)
                nops = []
                for w in waits[:-1]:
                    eng = eng_map[inst.engine]
                    bnop = eng.isa(
                        nc.isa.Opcode.NEURON_ISA_TPB_OPCODE_ENGINE_NOP, {}
                    )
                    ni = bnop.ins
                    # engine_nop appended to the current bb; remove it
                    removed = False
                    for f2 in nc.m.functions:
                        for bb2 in f2.blocks:
                            l2 = bb2.instructions
                            if l2 and l2[-1] is ni:
                                l2.pop()
                                removed = True
                                break
                        if removed:
                            break
                    assert removed, "could not relocate wait nop"
                    ni.sync_info = mybir.SyncInfo(on_wait=[w], on_update=[])
                    nops.append(ni)
                inst.sync_info = mybir.SyncInfo(
                    on_wait=[waits[-1]], on_update=list(si.on_update)
                )
                idx = None
                for j in range(len(lst)):
                    if lst[j] is inst:
                        idx = j
                        break
                assert idx is not None
                for k, ni in enumerate(nops):
                    lst.insert(idx + k, ni)
    return nc


def _build_bass(reps=1):
    # Bacc (not plain Bass): its finalize() pipeline runs
    # generate_event_semaphores, which splits Tile's multi-wait sync into the
    # 1-wait-per-instruction form this walrus requires.
    nc = bacc.Bacc("TRN2", target_bir_lowering=False)

    # ---- DRAM I/O ----
    QK8, VT8, OUT8 = CFG["fp8_qk"], CFG["fp8_vt"], CFG["fp8_out"]
    QKD = FP8 if QK8 else BF16
    VTD = FP8 if VT8 else BF16
    OUTD = FP8 if OUT8 else BF16
    x_d = nc.dram_tensor("x", [DIM, NTOK], BF16, kind="ExternalInput")
    # q,k weights (x SW_W if fp8), ln_w folded in: (c, m) m in [0, 2048)
    wqk_d = nc.dram_tensor("wqk", [DIM, 2 * DIM], QKD, kind="ExternalInput")
    # v weights: (c, m) m in [0, 1024)
    wv_d = nc.dram_tensor("wv", [DIM, DIM], VTD, kind="ExternalInput")
    # out-proj weights w_out.T: (c, c_out)
    wo_d = nc.dram_tensor("wo", [DIM, DIM], OUTD, kind="ExternalInput")
    # rank-1 LN-mean correction rows: a[m] = sum_c W'[c, m], as fp8
    # DoubleRow pairs with a zero second k-tile (half the PE row cost)
    aqk_d = nc.dram_tensor("aqk", [1, 2, 2 * DIM], FP8, kind="ExternalInput")
    avbv_d = nc.dram_tensor("avbv", [1, 2, DIM], FP8, kind="ExternalInput")
    # biases (from ln_b): per (partition, chunk) for q,k; row for v
    bqk_d = nc.dram_tensor("bqk", [128, 16], F32, kind="ExternalInput")
    # rope tables x s_qk, 2 heads stacked (128, 128); r-variants unscaled
    cosb_d = nc.dram_tensor("cosb", [128, WIN], BF16, kind="ExternalInput")
    sinb_d = nc.dram_tensor("sinb", [128, WIN], BF16, kind="ExternalInput")
    # rotate-half matrix (lhsT), block-diag for 2 heads
    st_d = nc.dram_tensor("st128", [128, 128], BF16, kind="ExternalInput")
    # eye-columns for denominator accumulation: E[:, h, m] = (m == h)
    e16_d = nc.dram_tensor("e16", [128, HEADS, HEADS], BF16, kind="ExternalInput")
    sel_d = nc.dram_tensor("sel", [HEADS, KC, 128], BF16, kind="ExternalInput") if not CFG["sel_dma"] else None
    ones_col_d = nc.dram_tensor("ones_col", [128, 1], BF16, kind="ExternalInput")
    ones2_d = nc.dram_tensor("ones2", [128, 2, 32], FP8, kind="ExternalInput")
    ones_row_d = nc.dram_tensor("ones_row", [1, 128], BF16, kind="ExternalInput")
    out_d = nc.dram_tensor("out", [DIM, NTOK], BF16, kind="ExternalOutput")

    x_r = x_d.ap().rearrange("(kc p) n -> p kc n", p=128)
    out_r = out_d.ap().rearrange("(kc p) n -> p kc n", p=128)

    with tile.TileContext(nc) as tc:
        with (
            tc.tile_pool(name="wpool", bufs=1) as wpool,
            tc.tile_pool(name="xpool", bufs=2) as xpool,
            tc.tile_pool(name="spool", bufs=2) as spool,
            tc.tile_pool(name="qkpool", bufs=1) as qkpool,
            tc.tile_pool(name="tpool", bufs=3) as tpool,
            tc.tile_pool(name="vpool", bufs=2) as vpool,
            tc.tile_pool(name="apool", bufs=2) as apool,
            tc.tile_pool(name="ypool", bufs=2) as ypool,
            tc.tile_pool(name="dpool", bufs=2, space="DRAM") as dpool,
            tc.tile_pool(name="psA", bufs=CFG["psa_bufs"], space="PSUM") as psA,
        ):
            # ---- resident weights/constants ----
            aqk = wpool.tile([1, 2, 2 * DIM], FP8)
            nc.sync.dma_start(out=aqk, in_=aqk_d.ap())
            avbv = wpool.tile([1, 2, DIM], FP8)
            nc.sync.dma_start(out=avbv, in_=avbv_d.ap())
            bqk = wpool.tile([128, 16], F32)
            nc.sync.dma_start(out=bqk, in_=bqk_d.ap())
            cosb = wpool.tile([128, WIN], BF16)
            nc.sync.dma_start(out=cosb, in_=cosb_d.ap())
            sinb = wpool.tile([128, WIN], BF16)
            nc.sync.dma_start(out=sinb, in_=sinb_d.ap())
            st128 = wpool.tile([128, 128], BF16)
            nc.sync.dma_start(out=st128, in_=st_d.ap())
            e16 = wpool.tile([128, HEADS, HEADS], BF16)
            nc.sync.dma_start(out=e16, in_=e16_d.ap())
            if not CFG["sel_dma"]:
                sel = wpool.tile([HEADS, KC, 128], BF16)
                nc.sync.dma_start(out=sel, in_=sel_d.ap())
            ones_col = wpool.tile([128, 1], BF16)
            nc.sync.dma_start(out=ones_col, in_=ones_col_d.ap())
            ones2_f8 = wpool.tile([128, 2, 32], FP8)
            nc.sync.dma_start(out=ones2_f8, in_=ones2_d.ap())
            ones_row = wpool.tile([1, 128], BF16)
            nc.sync.dma_start(out=ones_row, in_=ones_row_d.ap())
            eps_t = wpool.tile([1, 1], F32)
            nc.vector.memset(eps_t, EPS)
            zero128 = wpool.tile([128, 1], F32)
            nc.vector.memset(zero128, 0.0)
            wqk = wpool.tile([128, KC, 2 * DIM], QKD)
            nc.sync.dma_start(out=wqk, in_=wqk_d.ap().rearrange("(kc p) m -> p kc m", p=128))
            wv = wpool.tile([128, KC, DIM], VTD)
            nc.sync.dma_start(out=wv, in_=wv_d.ap().rearrange("(kc p) m -> p kc m", p=128))
            wo = wpool.tile([128, KC, DIM], OUTD)
            nc.sync.dma_start(out=wo, in_=wo_d.ap().rearrange("(kc p) m -> p kc m", p=128))

            def bcast_win(ap_2d, nwin):
                """(128, WIN) tile -> (128, nwin, WIN) stride-0 repeat."""
                return bass.AP(
                    tensor=ap_2d.tensor,
                    offset=ap_2d.offset,
                    ap=[ap_2d.ap[0], [0, nwin], ap_2d.ap[1]],
                )

            def prologue_load(it):
                tb = it * NT
                # x arrives bf16; the tile stays resident through outproj's
                # residual add (bufs=3: three pipeline stages in flight)
                xb = xpool.tile([128, KC, NT], BF16, tag="xb", bufs=3,
                                name=f"xb_{it}")
                nc.sync.dma_start(out=xb, in_=x_r[:, :, tb : tb + NT])
                return dict(it=it, tb=tb, xb=xb)

            def prologue(st0):
                it, tb, xb = st0["it"], st0["tb"], st0["xb"]
                _mark(nc, f'ln_stats_{it}')
                # LN stats: sum(x), sum(x^2) over channels via PE
                ps_sum = psA.tile([1, NT], F32, tag="mm1", name=f"ps_sum_{it}")
                ps_sq = psA.tile([1, NT], F32, tag="mm1", name=f"ps_sq_{it}")
                for kc in range(KC):
                    x2 = tpool.tile([128, NT], BF16, tag="tmp", name=f"x2_{it}_{kc}")
                    nc.vector.tensor_mul(out=x2, in0=xb[:, kc, :],
                                         in1=xb[:, kc, :])
                    nc.tensor.matmul(
                        ps_sum[:, :], ones_col, xb[:, kc, :],
                        start=(kc == 0), stop=(kc == KC - 1),
                    )
                    nc.tensor.matmul(
                        ps_sq[:, :], ones_col, x2,
                        start=(kc == 0), stop=(kc == KC - 1),
                    )
                ex = spool.tile([1, NT], F32, tag="sa", name=f"ex_{it}")
                nc.scalar.mul(out=ex, in_=ps_sum[:, :], mul=1.0 / DIM)
                ex2 = spool.tile([1, NT], F32, tag="sb", name=f"ex2_{it}")
                nc.scalar.mul(out=ex2, in_=ps_sq[:, :], mul=1.0 / DIM)
                negex2 = spool.tile([1, NT], F32, tag="sc", name=f"negex2_{it}")
                nc.vector.scalar_tensor_tensor(
                    out=negex2, in0=ex, scalar=-1.0, in1=ex,
                    op0=mybir.AluOpType.mult, op1=mybir.AluOpType.mult,
                )
                var = spool.tile([1, NT], F32, tag="sa", name=f"var_{it}")
                nc.vector.tensor_add(out=var, in0=ex2, in1=negex2)
                sd = spool.tile([1, NT], F32, tag="sc", name=f"sd_{it}")
                nc.scalar.activation(
                    out=sd, in_=var, func=mybir.ActivationFunctionType.Sqrt,
                    bias=eps_t[:, :], scale=1.0,
                )
                rstd = spool.tile([1, NT], F32, tag="sb", name=f"rstd_{it}")
                nc.vector.reciprocal_approx_fast(out=rstd, in_=sd)
                nmr = spool.tile([1, NT], F32, tag="sc", name=f"nmr_{it}")
                nc.vector.scalar_tensor_tensor(
                    out=nmr, in0=ex, scalar=-1.0, in1=rstd,
                    op0=mybir.AluOpType.mult, op1=mybir.AluOpType.mult,
                )
                # rb carries rstd * SW_X so x_s lands pre-scaled (descaled in
                # the PSUM->SBUF copies)
                rstd_bf = spool.tile([1, NT], BF16, tag="sbf", bufs=4,
                                     name=f"rstd_bf_{it}")
                nc.scalar.mul(out=rstd_bf, in_=rstd, mul=SW_X)
                # fp8 DoubleRow pair row: k-tile0 = -mu*rstd*SW_X,
                # k-tile1 = 0 (memset-0 fp8 is safe; memset-1.0 fp8 is not)
                nmr2 = spool.tile([1, 2, NT], FP8, tag="sbf", bufs=4,
                                  name=f"nmr2_{it}")
                nc.vector.memset(nmr2, 0.0)
                nc.scalar.mul(out=nmr2[:, 0, :], in_=nmr, mul=SW_X)
                # broadcast rstd to all partitions via a DRAM bounce (keeps
                # Pool/PE out of the serial LN-stats chain; stride-0 partition
                # reads are only legal from DRAM)
                rb = spool.tile([128, NT], BF16, tag="rb", name=f"rb_{it}")
                rdram = dpool.tile([1, NT], BF16, tag="rdram", name=f"rdram_{it}")
                nc.sync.dma_start(out=rdram, in_=rstd_bf)
                rstd_bcast_src = bass.AP(
                    tensor=rdram.tensor, offset=rdram.offset,
                    ap=[[0, 128], rdram.ap[1]],
                )
                nc.sync.dma_start(out=rb, in_=rstd_bcast_src)
                # x_s = xb * rstd * SW_X (mean handled by rank-1 matmul
                # downstream)
                rb_b = bass.AP(tensor=rb.tensor, offset=rb.offset,
                               ap=[rb.ap[0], [0, KC], rb.ap[1]])
                x_s = xpool.tile([128, KC, NT], QKD, tag="xs", name=f"x_s_{it}")
                nc.vector.tensor_mul(out=x_s, in0=xb, in1=rb_b)
                x_sv = x_s
                return dict(it=it, tb=tb, x_s=x_s, x_sv=x_sv, nmr2=nmr2,
                            xb=xb)

            def qkvt_phase(st):
                it, x_s, x_sv = st["it"], st["x_s"], st["x_sv"]
                nmr2 = st["nmr2"]
                _mark(nc, f'qkrope_{it}')
                qk = qkpool.tile([128, 16, NT], BF16, tag="qk", bufs=1,
                                 name=f"qk_{it}")
                vt = vpool.tile([128, WPT, DIM], BF16, tag="vt", bufs=2,
                                name=f"vt_{it}")
                s_qk = 1.0 / (SW if QK8 else SW_X)
                s_vt = 1.0 / (SW if VT8 else SW_X)
                pend = []

                def rope_tail(mc, qsin, qcos):
                    ps_u = psA.tile([128, NT], F32, tag="mm1",
                                    name=f"ps_u_{it}_{mc}")
                    nc.tensor.matmul(ps_u[:, :], st128, qsin,
                                     start=True, stop=True)
                    nc.vector.tensor_add(out=qk[:, mc, :], in0=ps_u[:, :],
                                         in1=qcos)

                def qk_chunk(mc):
                    ps_qk = psA.tile([128, NT], F32, tag="mm1",
                                     name=f"ps_qk_{it}_{mc}")
                    if QK8:
                        for p2 in range(KC // 2):
                            for nh in range(2):
                                ncol = slice(nh * 256, (nh + 1) * 256)
                                nc.tensor.matmul(
                                    ps_qk[:, ncol],
                                    wqk[:, 2 * p2 : 2 * p2 + 2,
                                        mc * 128 : (mc + 1) * 128],
                                    x_s[:, 2 * p2 : 2 * p2 + 2, ncol],
                                    start=(p2 == 0),
                                    stop=False,
                                    perf_mode=DR,
                                    skip_group_check=True,
                                )
                    else:
                        for kc in range(KC):
                            nc.tensor.matmul(
                                ps_qk[:, :],
                                wqk[:, kc, mc * 128 : (mc + 1) * 128],
                                x_s[:, kc, :],
                                start=(kc == 0), stop=False,
                            )
                    for h2 in range(2):
                        nc.tensor.matmul(
                            ps_qk[:, h2 * 256 : (h2 + 1) * 256],
                            aqk[:, :, mc * 128 : (mc + 1) * 128],
                            nmr2[:, :, h2 * 256 : (h2 + 1) * 256],
                            start=False, stop=True,
                            perf_mode=DR,
                            skip_group_check=True,
                        )
                    qsin = tpool.tile([128, NT], BF16, tag="tsin", bufs=2,
                                      name=f"qsin_{it}_{mc}")
                    nc.vector.scalar_tensor_tensor(
                        out=qsin, in0=ps_qk[:, :], scalar=bqk[:, mc : mc + 1],
                        in1=bcast_win(sinb, WPT),
                        op0=mybir.AluOpType.add, op1=mybir.AluOpType.mult,
                    )
                    qcos = tpool.tile([128, NT], BF16, tag="tcos", bufs=2,
                                      name=f"qcos_{it}_{mc}")
                    nc.vector.scalar_tensor_tensor(
                        out=qcos, in0=ps_qk[:, :], scalar=bqk[:, mc : mc + 1],
                        in1=bcast_win(cosb, WPT),
                        op0=mybir.AluOpType.add, op1=mybir.AluOpType.mult,
                    )
                    pend.append((mc, qsin, qcos))
                    if len(pend) > ROPE_LAG:
                        rope_tail(*pend.pop(0))

                def vt_group(g):
                    sub, nh = g // 2, g % 2
                    ncol = slice(nh * 512, (nh + 1) * 512)
                    ps_vt = psA.tile([128, NT], F32, tag="vtp", bufs=2,
                                     name=f"ps_vt_{it}_{sub}_{nh}")
                    if VT8:
                        for p2 in range(KC // 2):
                            for qh in range(2):
                                qcol = slice(nh * 512 + qh * 256,
                                             nh * 512 + (qh + 1) * 256)
                                nc.tensor.matmul(
                                    ps_vt[:, qh * 256 : (qh + 1) * 256],
                                    x_sv[:, 2 * p2 : 2 * p2 + 2,
                                         sub * 128 : (sub + 1) * 128],
                                    wv[:, 2 * p2 : 2 * p2 + 2, qcol],
                                    start=(p2 == 0),
                                    stop=False,
                                    perf_mode=DR,
                                    skip_group_check=True,
                                )
                    else:
                        for kc in range(KC):
                            nc.tensor.matmul(
                                ps_vt[:, :],
                                x_sv[:, kc, sub * 128 : (sub + 1) * 128],
                                wv[:, kc, ncol],
                                start=(kc == 0), stop=False,
                            )
                    for h2 in range(2):
                        nc.tensor.matmul(
                            ps_vt[:, h2 * 256 : (h2 + 1) * 256],
                            nmr2[:, :, sub * 128 : (sub + 1) * 128],
                            avbv[:, :, nh * 512 + h2 * 256 : nh * 512 + (h2 + 1) * 256],
                            start=False, stop=True,
                            perf_mode=DR,
                            skip_group_check=True,
                        )
                    if g % 2 == 0:
                        nc.scalar.activation(
                            out=vt[:, sub, ncol],
                            in_=ps_vt[:, :],
                            func=mybir.ActivationFunctionType.Identity,
                            bias=zero128[:, :], scale=s_vt,
                        )
                    else:
                        nc.vector.tensor_scalar(
                            out=vt[:, sub, ncol],
                            in0=ps_vt[:, :], scalar1=s_vt, scalar2=None,
                            op0=mybir.AluOpType.mult,
                        )

                for mc in range(16):
                    qk_chunk(mc)
                    if mc >= 8:
                        vt_group(mc - 8)
                while pend:
                    rope_tail(*pend.pop(0))
                st["qk"] = qk
                st["vt"] = vt

            def attn_pass1(st):
                it, qk = st["it"], st["qk"]
                _mark(nc, f'attn_{it}')
                expt = apool.tile([128, WPT, 2, 8 * WIN], BF16, tag="expt",
                                  bufs=1, name=f"expt_{it}")
                ps_d = psA.tile([HEADS, WPT * WIN], F32, tag="dd", bufs=1,
                                name=f"ps_d_{it}")

                def ecol(hh):
                    return (hh % 2) * 512 + (hh // 2) * WIN

                def scores(wl):
                    wslc = slice(wl * WIN, (wl + 1) * WIN)
                    # parity-split so matmuls with different contraction
                    # row-groups (operand base partition 0 vs 64) never share
                    # a PSUM bank (HW faults otherwise).
                    for hg in range(2):
                        for par in range(2):
                            ps_sc = psA.tile([128, 4 * WIN], F32, tag="mm1",
                                             name=f"ps_sc_{it}_{wl}_{hg}_{par}")
                            po = par * 64
                            for j in range(4):
                                h = hg * 8 + 2 * j + par
                                qh = qk[po : po + 64, h // 2, wslc]
                                kh = qk[po : po + 64, 8 + h // 2, wslc]
                                nc.tensor.matmul(
                                    ps_sc[:, j * WIN : (j + 1) * WIN],
                                    kh, qh, start=True, stop=True,
                                )
                            nc.scalar.activation(
                                out=expt[:, wl, hg, par * 512 : (par + 1) * 512],
                                in_=ps_sc[:, :],
                                func=mybir.ActivationFunctionType.Exp,
                                bias=zero128[:, :], scale=0.125,
                            )

                def denom(wl):
                    for hg in range(2):
                        for hh in range(8):
                            h = hg * 8 + hh
                            nc.tensor.matmul(
                                ps_d[:, wl * WIN : (wl + 1) * WIN],
                                e16[:, h, :],
                                expt[:, wl, hg, ecol(hh) : ecol(hh) + WIN],
                                start=(h == 0), stop=(h == HEADS - 1),
                            )

                # all scores first (releases the single-buffered qk tile
                # early for the next tile's rope writes), then denominators
                # (their expt inputs are ready by then)
                for wl in range(WPT):
                    scores(wl)
                for wl in range(WPT):
                    denom(wl)

                rd = spool.tile([HEADS, WPT * WIN], F32, tag="rd", bufs=1,
                                name=f"rd_{it}")
                nc.vector.reciprocal_approx_fast(out=rd, in_=ps_d[:, :])
                rd_bf = spool.tile([HEADS, WPT * WIN], BF16, tag="rdb", bufs=1,
                                   name=f"rd_bf_{it}")
                nc.scalar.mul(out=rd_bf, in_=rd,
                              mul=SW_X if OUT8 else 1.0)
                # head-major flat bounce: head h's 4-window row at offset
                # h*512; partition half ph reads heads 2c+ph (stride 1024)
                bcw = apool.tile([128, KC, WPT, WIN], BF16, tag="bc",
                                 bufs=1, name=f"bcw_{it}")
                rd_dram = dpool.tile([HEADS, WPT * WIN], BF16, tag="rd_dram",
                                     name=f"rd_dram_{it}")
                nc.sync.dma_start(out=rd_dram, in_=rd_bf)
                for ph in range(2):
                    srcap = bass.AP(
                        tensor=rd_dram.tensor,
                        offset=rd_dram.offset + ph * (WPT * WIN),
                        ap=[[0, 64], [2 * WPT * WIN, KC], [1, WPT * WIN]],
                    )
                    nc.sync.dma_start(
                        out=bcw[ph * 64 : (ph + 1) * 64], in_=srcap)
                st["expt"] = expt
                st["bcw"] = bcw

            def attn_pass2(st):
                it, vt, expt, bcw = st["it"], st["vt"], st["expt"], st["bcw"]
                attn_t = apool.tile([128, KC, NT], OUTD, tag="attn", bufs=1,
                                    name=f"attn_t_{it}")

                def ecol2(hh):
                    return (hh % 2) * 512 + (hh // 2) * WIN

                for wl in range(WPT):
                    wslc = slice(wl * WIN, (wl + 1) * WIN)
                    for hg in range(2):
                        ps_at = psA.tile([128, 4 * WIN], F32, tag="mm1",
                                         name=f"ps_at_{it}_{wl}_{hg}")
                        for hh in range(8):
                            h = hg * 8 + hh
                            po = (h % 2) * 64
                            c = (h // 2) % 4
                            nc.tensor.matmul(
                                ps_at[po : po + 64, c * WIN : (c + 1) * WIN],
                                vt[:, wl, h * 64 : (h + 1) * 64],
                                expt[:, wl, hg, ecol2(hh) : ecol2(hh) + WIN],
                                start=True, stop=True,
                                tile_position=(0, po),
                            )
                        nc.vector.tensor_mul(
                            out=attn_t[:, 4 * hg : 4 * hg + 4, wslc],
                            in0=ps_at.rearrange("p (c i) -> p c i", c=4),
                            in1=bcw[:, 4 * hg : 4 * hg + 4, wl, :],
                        )
                st["attn_t"] = attn_t

            def outproj_phase(st):
                it, tb, attn_t, xb = st["it"], st["tb"], st["attn_t"], st["xb"]
                _mark(nc, f'outproj_{it}')
                s_out = 1.0 / SW if OUT8 else 1.0
                for mc in range(KC):
                    ps_y = psA.tile([128, NT], F32, tag="mm1",
                                    name=f"ps_y_{it}_{mc}")
                    if OUT8:
                        for p2 in range(KC // 2):
                            for nh in range(2):
                                ncol = slice(nh * 256, (nh + 1) * 256)
                                nc.tensor.matmul(
                                    ps_y[:, ncol],
                                    wo[:, 2 * p2 : 2 * p2 + 2,
                                       mc * 128 : (mc + 1) * 128],
                                    attn_t[:, 2 * p2 : 2 * p2 + 2, ncol],
                                    start=(p2 == 0),
                                    stop=(p2 == KC // 2 - 1),
                                    perf_mode=DR,
                                    skip_group_check=True,
                                )
                    else:
                        for kc in range(KC):
                            nc.tensor.matmul(
                                ps_y[:, :],
                                wo[:, kc, mc * 128 : (mc + 1) * 128],
                                attn_t[:, kc, :],
                                start=(kc == 0), stop=(kc == KC - 1),
                            )
                    y = ypool.tile([128, NT], BF16, tag="y", bufs=3,
                                   name=f"y_{it}_{mc}")
                    nc.vector.scalar_tensor_tensor(
                        out=y, in0=ps_y[:, :], scalar=s_out, in1=xb[:, mc, :],
                        op0=mybir.AluOpType.mult, op1=mybir.AluOpType.add,
                    )
                    nc.sync.dma_start(
                        out=out_r[:, mc, tb : tb + NT], in_=y,
                    )

            # software pipeline: tile i+1's prologue+qkrope+vt are emitted
            # between tile i's attention and outproj, so PE's in-order stream
            # has GEMM work to chew on while attention waits on the softmax
            # denominator round-trip
            its = [t for _ in range(reps) for t in range(NTILES)]
            loads = {0: prologue_load(its[0])}
            if len(its) > 1:
                loads[1] = prologue_load(its[1])
            states = {0: prologue(loads.pop(0))}
            qkvt_phase(states[0])
            for idx, it in enumerate(its):
                st = states.pop(idx)
                if idx + 2 < len(its):
                    loads[idx + 2] = prologue_load(its[idx + 2])
                if idx + 1 < len(its):
                    states[idx + 1] = prologue(loads.pop(idx + 1))
                attn_pass1(st)
                attn_pass2(st)
                if idx + 1 < len(its):
                    qkvt_phase(states[idx + 1])
                outproj_phase(st)
    nc.finalize()
    return nc


def _host_prep(x, ln_w, ln_b, w_qkv, w_out):
    """Shared (non-x) device inputs, host-precomputed."""
    bf = ml_dtypes.bfloat16
    f8 = ml_dtypes.float8_e4m3fn
    wqkv_s = (w_qkv * ln_w[None, :]).astype(np.float32)  # (3C, C) scaled
    wT = np.ascontiguousarray(wqkv_s.T)  # (C, 3C)
    b_qkv = (w_qkv @ ln_b).astype(np.float32)  # (3C,)
    a_qkv = wqkv_s.sum(axis=1).astype(np.float32)  # (3C,)

    QK8, VT8, OUT8 = CFG["fp8_qk"], CFG["fp8_vt"], CFG["fp8_out"]
    ins = {}
    ins["wqk"] = (
        np.ascontiguousarray(wT[:, : 2 * DIM]) * (SW_W if QK8 else 1.0)
    ).astype(f8 if QK8 else bf)
    ins["wv"] = (
        np.ascontiguousarray(wT[:, 2 * DIM :]) * (SW_W if VT8 else 1.0)
    ).astype(f8 if VT8 else bf)
    ins["wo"] = (
        np.ascontiguousarray(w_out.T) * (SW_W if OUT8 else 1.0)
    ).astype(f8 if OUT8 else bf)
    aqk2 = np.zeros((1, 2, 2 * DIM), np.float32)
    aqk2[0, 0, :] = a_qkv[: 2 * DIM]
    ins["aqk"] = aqk2.astype(f8)
    av2 = np.zeros((1, 2, DIM), np.float32)
    av2[0, 0, :] = a_qkv[2 * DIM :]
    ins["avbv"] = av2.astype(f8)
    # q,k bias as (partition, chunk), divided by s_qk (the rope tables carry
    # s_qk, so (ps + b/s)*(t*s) = (ps*s + b)*t)
    ins["bqk"] = np.ascontiguousarray(
        b_qkv[: 2 * DIM].reshape(16, 128).T / (1.0 / (SW if QK8 else SW_X))
    ).astype(np.float32)

    s_qk = 1.0 / (SW if QK8 else SW_X)
    inv_freq = 1.0 / 10000 ** (np.arange(0, DHEAD, 2, dtype=np.float32) / DHEAD)
    pos = np.arange(WIN, dtype=np.float32)
    freqs = np.concatenate([np.outer(pos, inv_freq)] * 2, axis=-1)  # (WIN, 64)
    cos_t = np.cos(freqs).T.astype(np.float32) * s_qk  # (64, WIN)
    sin_t = np.sin(freqs).T.astype(np.float32) * s_qk
    ins["cosb"] = np.tile(cos_t, (2, 1)).astype(bf)
    ins["sinb"] = np.tile(sin_t, (2, 1)).astype(bf)

    S = np.zeros((DHEAD, DHEAD), np.float32)
    S[: DHEAD // 2, DHEAD // 2 :] = -np.eye(DHEAD // 2)
    S[DHEAD // 2 :, : DHEAD // 2] = np.eye(DHEAD // 2)
    ST = S.T
    st128 = np.zeros((128, 128), np.float32)
    st128[:64, :64] = ST
    st128[64:, 64:] = ST
    ins["st128"] = st128.astype(bf)

    e = np.zeros((128, HEADS, HEADS), np.float32)
    for h in range(HEADS):
        e[:, h, h] = 1.0
    ins["e16"] = e.astype(bf)


    ins["ones_col"] = np.ones((128, 1), np.float32).astype(bf)
    ins["ones2"] = np.ones((128, 2, 32), np.float32).astype(f8)
    ins["ones_row"] = np.ones((1, 128), np.float32).astype(bf)
    return ins


def kernel(x, ln_w, ln_b, w_qkv, w_out, _want_trace=False):
    x = np.asarray(x, dtype=np.float32)
    shared = _host_prep(
        np.asarray(x, np.float32),
        np.asarray(ln_w, np.float32),
        np.asarray(ln_b, np.float32),
        np.asarray(w_qkv, np.float32),
        np.asarray(w_out, np.float32),
    )

    if "nc" not in _CACHED:
        _CACHED["nc"] = _build_bass()
    nc = _CACHED["nc"]

    x_bf = x.astype(ml_dtypes.bfloat16)
    in_maps = []
    for core in range(NCORE):
        b, half = core // 2, core % 2
        xs = np.ascontiguousarray(x_bf[b, :, half * NTOK : (half + 1) * NTOK])
        m = dict(shared)
        m["x"] = xs
        in_maps.append(m)

    res = run_bass_kernel_spmd(
        nc, in_maps, core_ids=list(range(NCORE)), trace=_want_trace
    )
    out = np.empty((B, DIM, T), np.float32)
    for core in range(NCORE):
        b, half = core // 2, core % 2
        out[b, :, half * NTOK : (half + 1) * NTOK] = (
            res.results[core]["out"].astype(np.float32)
        )
    if _want_trace:
        _CACHED["last_trace"] = res
    return out



# revision 109
# speedup vs baseline: 1.0008x; 1.0008x over previous
"""LocalMHA (windowed attention) Trainium2 Bass kernel.

Full inputs -> full outputs. Internally: 8-way data-parallel over
(batch, token-half) shards; each NeuronCore runs the complete block on
4096 tokens (32 windows of 128). No collectives.

Problem (hardcoded):
  x: (4, 1024, 8192) f32, DIM=1024, HEADS=16, DIM_HEAD=64, WINDOW=128
  out = W_out @ attn(LN(x)) + x   (per reference.py)
"""

import numpy as np
import ml_dtypes

import concourse.bass as bass
import concourse.bacc as bacc
import concourse.tile as tile
from concourse import mybir
from concourse.bass_utils import run_bass_kernel_spmd

BF16 = mybir.dt.bfloat16
F32 = mybir.dt.float32
FP8 = mybir.dt.float8e4
DR = mybir.MatmulPerfMode.DoubleRow

# fp8 scale factors: weights x SW_W, activations x SW_X; PSUM carries
# SW_W*SW_X, descaled in the PSUM->SBUF copy.
SW_W = 32.0
SW_X = 16.0
SW = SW_W * SW_X

B, DIM, T = 4, 1024, 8192
HEADS, DHEAD, WIN = 16, 64, 128
NCORE = 8
NTOK = (B * T) // NCORE          # 4096 tokens per core
NT = 512                         # token tile
NTILES = NTOK // NT              # 8
KC = DIM // 128                  # 8 contraction chunks
WPT = NT // WIN                  # 4 windows per token tile
ROPE_LAG = 2                     # chunks of lag before rope S-matmul
EPS = 1e-5

_CACHED = {}
PHASE_LOG = []

# build-time tuning knobs (swept via TimelineSim)
CFG = dict(
    sel_dma=True,       # denom broadcast via DRAM-bounce DMA vs sel matmuls
    sq_gpsimd=False,    # x^2 on gpsimd vs ACT
    psa_bufs=5,
    psb_bufs=1,
    have_bias=False,    # ln_b != 0: emit bias rank-1 in the vT path
    fp8_qk=False,       # q,k projection GEMM in fp8 DoubleRow
    fp8_vt=False,       # v projection GEMM in fp8 DoubleRow
    fp8_out=False,      # out projection GEMM in fp8 DoubleRow
)


def _mark(nc, phase):
    PHASE_LOG.append((phase, len(nc.inst_map)))


def _legalize_waits(nc):
    """This toolchain's walrus encodes at most ONE sem-wait per instruction
    (ISA EVENTS struct has a single wait slot) and errors with 'Too many sync
    wait commands' on Tile's multi-wait output. Split: hoist all but one wait
    onto same-engine ENGINE_NOPs inserted immediately before the instruction
    (engine stalls there first -> identical ordering semantics)."""
    eng_map = {
        mybir.EngineType.PE: nc.tensor,
        mybir.EngineType.Activation: nc.scalar,
        mybir.EngineType.DVE: nc.vector,
        mybir.EngineType.Pool: nc.gpsimd,
        mybir.EngineType.SP: nc.sync,
    }
    for f in nc.m.functions:
        for bb in f.blocks:
            lst = bb.instructions  # live list
            need = [
                i for i in lst
                if i.sync_info is not None and len(i.sync_info.on_wait) > 1
            ]
            for inst in need:
                si = inst.sync_info
                waits = list(si.on_wait)
                nops = []
                for w in waits[:-1]:
                    eng = eng_map[inst.engine]
                    bnop = eng.isa(
                        nc.isa.Opcode.NEURON_ISA_TPB_OPCODE_ENGINE_NOP, {}
                    )
                    ni = bnop.ins
                    # engine_nop appended to the current bb; remove it
                    removed = False
                    for f2 in nc.m.functions:
                        for bb2 in f2.blocks:
                            l2 = bb2.instructions
                            if l2 and l2[-1] is ni:
                                l2.pop()
                                removed = True
                                break
                        if removed:
                            break
                    assert removed, "could not relocate wait nop"
                    ni.sync_info = mybir.SyncInfo(on_wait=[w], on_update=[])
                    nops.append(ni)
                inst.sync_info = mybir.SyncInfo(
                    on_wait=[waits[-1]], on_update=list(si.on_update)
                )
                idx = None
                for j in range(len(lst)):
                    if lst[j] is inst:
                        idx = j
                        break
                assert idx is not None
                for k, ni in enumerate(nops):
                    lst.insert(idx + k, ni)
    return nc


def _build_bass(reps=1):
    # Bacc (not plain Bass): its finalize() pipeline runs
    # generate_event_semaphores, which splits Tile's multi-wait sync into the
    # 1-wait-per-instruction form this walrus requires.
    nc = bacc.Bacc("TRN2", target_bir_lowering=False)

    # ---- DRAM I/O ----
    QK8, VT8, OUT8 = CFG["fp8_qk"], CFG["fp8_vt"], CFG["fp8_out"]
    QKD = FP8 if QK8 else BF16
    VTD = FP8 if VT8 else BF16
    OUTD = FP8 if OUT8 else BF16
    x_d = nc.dram_tensor("x", [DIM, NTOK], BF16, kind="ExternalInput")
    # q,k weights (x SW_W if fp8), ln_w folded in: (c, m) m in [0, 2048)
    wqk_d = nc.dram_tensor("wqk", [DIM, 2 * DIM], QKD, kind="ExternalInput")
    # v weights: (c, m) m in [0, 1024)
    wv_d = nc.dram_tensor("wv", [DIM, DIM], VTD, kind="ExternalInput")
    # out-proj weights w_out.T: (c, c_out)
    wo_d = nc.dram_tensor("wo", [DIM, DIM], OUTD, kind="ExternalInput")
    # rank-1 LN-mean correction rows: a[m] = sum_c W'[c, m], as fp8
    # DoubleRow pairs with a zero second k-tile (half the PE row cost)
    aqk_d = nc.dram_tensor("aqk", [1, 2, 2 * DIM], FP8, kind="ExternalInput")
    avbv_d = nc.dram_tensor("avbv", [1, 2, DIM], FP8, kind="ExternalInput")
    # biases (from ln_b): per (partition, chunk) for q,k; row for v
    bqk_d = nc.dram_tensor("bqk", [128, 16], F32, kind="ExternalInput")
    # rope tables x s_qk, 2 heads stacked (128, 128); r-variants unscaled
    cosb_d = nc.dram_tensor("cosb", [128, WIN], BF16, kind="ExternalInput")
    sinb_d = nc.dram_tensor("sinb", [128, WIN], BF16, kind="ExternalInput")
    # rotate-half matrix (lhsT), block-diag for 2 heads
    st_d = nc.dram_tensor("st128", [128, 128], BF16, kind="ExternalInput")
    # eye-columns for denominator accumulation: E[:, h, m] = (m == h)
    e16_d = nc.dram_tensor("e16", [128, HEADS, HEADS], BF16, kind="ExternalInput")
    sel_d = nc.dram_tensor("sel", [HEADS, KC, 128], BF16, kind="ExternalInput") if not CFG["sel_dma"] else None
    ones_col_d = nc.dram_tensor("ones_col", [128, 1], BF16, kind="ExternalInput")
    ones_row_d = nc.dram_tensor("ones_row", [1, 128], BF16, kind="ExternalInput")
    out_d = nc.dram_tensor("out", [DIM, NTOK], BF16, kind="ExternalOutput")

    x_r = x_d.ap().rearrange("(kc p) n -> p kc n", p=128)
    out_r = out_d.ap().rearrange("(kc p) n -> p kc n", p=128)

    with tile.TileContext(nc) as tc:
        with (
            tc.tile_pool(name="wpool", bufs=1) as wpool,
            tc.tile_pool(name="xpool", bufs=2) as xpool,
            tc.tile_pool(name="spool", bufs=2) as spool,
            tc.tile_pool(name="qkpool", bufs=1) as qkpool,
            tc.tile_pool(name="tpool", bufs=3) as tpool,
            tc.tile_pool(name="vpool", bufs=2) as vpool,
            tc.tile_pool(name="apool", bufs=2) as apool,
            tc.tile_pool(name="ypool", bufs=2) as ypool,
            tc.tile_pool(name="dpool", bufs=2, space="DRAM") as dpool,
            tc.tile_pool(name="psA", bufs=CFG["psa_bufs"], space="PSUM") as psA,
        ):
            # ---- resident weights/constants ----
            aqk = wpool.tile([1, 2, 2 * DIM], FP8)
            nc.sync.dma_start(out=aqk, in_=aqk_d.ap())
            avbv = wpool.tile([1, 2, DIM], FP8)
            nc.sync.dma_start(out=avbv, in_=avbv_d.ap())
            bqk = wpool.tile([128, 16], F32)
            nc.sync.dma_start(out=bqk, in_=bqk_d.ap())
            cosb = wpool.tile([128, WIN], BF16)
            nc.sync.dma_start(out=cosb, in_=cosb_d.ap())
            sinb = wpool.tile([128, WIN], BF16)
            nc.sync.dma_start(out=sinb, in_=sinb_d.ap())
            st128 = wpool.tile([128, 128], BF16)
            nc.sync.dma_start(out=st128, in_=st_d.ap())
            e16 = wpool.tile([128, HEADS, HEADS], BF16)
            nc.sync.dma_start(out=e16, in_=e16_d.ap())
            if not CFG["sel_dma"]:
                sel = wpool.tile([HEADS, KC, 128], BF16)
                nc.sync.dma_start(out=sel, in_=sel_d.ap())
            ones_col = wpool.tile([128, 1], BF16)
            nc.sync.dma_start(out=ones_col, in_=ones_col_d.ap())
            ones_row = wpool.tile([1, 128], BF16)
            nc.sync.dma_start(out=ones_row, in_=ones_row_d.ap())
            eps_t = wpool.tile([1, 1], F32)
            nc.vector.memset(eps_t, EPS)
            zero128 = wpool.tile([128, 1], F32)
            nc.vector.memset(zero128, 0.0)
            wqk = wpool.tile([128, KC, 2 * DIM], QKD)
            nc.sync.dma_start(out=wqk, in_=wqk_d.ap().rearrange("(kc p) m -> p kc m", p=128))
            wv = wpool.tile([128, KC, DIM], VTD)
            nc.sync.dma_start(out=wv, in_=wv_d.ap().rearrange("(kc p) m -> p kc m", p=128))
            wo = wpool.tile([128, KC, DIM], OUTD)
            nc.sync.dma_start(out=wo, in_=wo_d.ap().rearrange("(kc p) m -> p kc m", p=128))

            def bcast_win(ap_2d, nwin):
                """(128, WIN) tile -> (128, nwin, WIN) stride-0 repeat."""
                return bass.AP(
                    tensor=ap_2d.tensor,
                    offset=ap_2d.offset,
                    ap=[ap_2d.ap[0], [0, nwin], ap_2d.ap[1]],
                )

            def prologue_load(it):
                tb = it * NT
                # x arrives bf16; the tile stays resident through outproj's
                # residual add (bufs=3: three pipeline stages in flight)
                xb = xpool.tile([128, KC, NT], BF16, tag="xb", bufs=3,
                                name=f"xb_{it}")
                nc.sync.dma_start(out=xb, in_=x_r[:, :, tb : tb + NT])
                return dict(it=it, tb=tb, xb=xb)

            def prologue(st0):
                it, tb, xb = st0["it"], st0["tb"], st0["xb"]
                _mark(nc, f'ln_stats_{it}')
                # LN stats: sum(x), sum(x^2) over channels via PE
                ps_sum = psA.tile([1, NT], F32, tag="mm1", name=f"ps_sum_{it}")
                ps_sq = psA.tile([1, NT], F32, tag="mm1", name=f"ps_sq_{it}")
                for kc in range(KC):
                    x2 = tpool.tile([128, NT], BF16, tag="tmp", name=f"x2_{it}_{kc}")
                    nc.vector.tensor_mul(out=x2, in0=xb[:, kc, :],
                                         in1=xb[:, kc, :])
                    nc.tensor.matmul(
                        ps_sum[:, :], ones_col, xb[:, kc, :],
                        start=(kc == 0), stop=(kc == KC - 1),
                    )
                    nc.tensor.matmul(
                        ps_sq[:, :], ones_col, x2,
                        start=(kc == 0), stop=(kc == KC - 1),
                    )
                ex = spool.tile([1, NT], F32, tag="sa", name=f"ex_{it}")
                nc.scalar.mul(out=ex, in_=ps_sum[:, :], mul=1.0 / DIM)
                ex2 = spool.tile([1, NT], F32, tag="sb", name=f"ex2_{it}")
                nc.scalar.mul(out=ex2, in_=ps_sq[:, :], mul=1.0 / DIM)
                negex2 = spool.tile([1, NT], F32, tag="sc", name=f"negex2_{it}")
                nc.vector.scalar_tensor_tensor(
                    out=negex2, in0=ex, scalar=-1.0, in1=ex,
                    op0=mybir.AluOpType.mult, op1=mybir.AluOpType.mult,
                )
                var = spool.tile([1, NT], F32, tag="sa", name=f"var_{it}")
                nc.vector.tensor_add(out=var, in0=ex2, in1=negex2)
                sd = spool.tile([1, NT], F32, tag="sc", name=f"sd_{it}")
                nc.scalar.activation(
                    out=sd, in_=var, func=mybir.ActivationFunctionType.Sqrt,
                    bias=eps_t[:, :], scale=1.0,
                )
                rstd = spool.tile([1, NT], F32, tag="sb", name=f"rstd_{it}")
                nc.vector.reciprocal_approx_fast(out=rstd, in_=sd)
                nmr = spool.tile([1, NT], F32, tag="sc", name=f"nmr_{it}")
                nc.vector.scalar_tensor_tensor(
                    out=nmr, in0=ex, scalar=-1.0, in1=rstd,
                    op0=mybir.AluOpType.mult, op1=mybir.AluOpType.mult,
                )
                # rb carries rstd * SW_X so x_s lands pre-scaled (descaled in
                # the PSUM->SBUF copies)
                rstd_bf = spool.tile([1, NT], BF16, tag="sbf", bufs=4,
                                     name=f"rstd_bf_{it}")
                nc.scalar.mul(out=rstd_bf, in_=rstd, mul=SW_X)
                # fp8 DoubleRow pair row: k-tile0 = -mu*rstd*SW_X,
                # k-tile1 = 0 (memset-0 fp8 is safe; memset-1.0 fp8 is not)
                nmr2 = spool.tile([1, 2, NT], FP8, tag="sbf", bufs=4,
                                  name=f"nmr2_{it}")
                nc.vector.memset(nmr2, 0.0)
                nc.scalar.mul(out=nmr2[:, 0, :], in_=nmr, mul=SW_X)
                # broadcast rstd to all partitions via a DRAM bounce (keeps
                # Pool/PE out of the serial LN-stats chain; stride-0 partition
                # reads are only legal from DRAM)
                rb = spool.tile([128, NT], BF16, tag="rb", name=f"rb_{it}")
                rdram = dpool.tile([1, NT], BF16, tag="rdram", name=f"rdram_{it}")
                nc.sync.dma_start(out=rdram, in_=rstd_bf)
                rstd_bcast_src = bass.AP(
                    tensor=rdram.tensor, offset=rdram.offset,
                    ap=[[0, 128], rdram.ap[1]],
                )
                nc.sync.dma_start(out=rb, in_=rstd_bcast_src)
                # x_s = xb * rstd * SW_X (mean handled by rank-1 matmul
                # downstream)
                rb_b = bass.AP(tensor=rb.tensor, offset=rb.offset,
                               ap=[rb.ap[0], [0, KC], rb.ap[1]])
                x_s = xpool.tile([128, KC, NT], QKD, tag="xs", name=f"x_s_{it}")
                nc.vector.tensor_mul(out=x_s, in0=xb, in1=rb_b)
                x_sv = x_s
                return dict(it=it, tb=tb, x_s=x_s, x_sv=x_sv, nmr2=nmr2,
                            xb=xb)

            def qkvt_phase(st):
                it, x_s, x_sv = st["it"], st["x_s"], st["x_sv"]
                nmr2 = st["nmr2"]
                _mark(nc, f'qkrope_{it}')
                qk = qkpool.tile([128, 16, NT], BF16, tag="qk", bufs=1,
                                 name=f"qk_{it}")
                vt = vpool.tile([128, WPT, DIM], BF16, tag="vt", bufs=2,
                                name=f"vt_{it}")
                s_qk = 1.0 / (SW if QK8 else SW_X)
                s_vt = 1.0 / (SW if VT8 else SW_X)
                pend = []

                def rope_tail(mc, qsin, qcos):
                    ps_u = psA.tile([128, NT], F32, tag="mm1",
                                    name=f"ps_u_{it}_{mc}")
                    nc.tensor.matmul(ps_u[:, :], st128, qsin,
                                     start=True, stop=True)
                    nc.vector.tensor_add(out=qk[:, mc, :], in0=ps_u[:, :],
                                         in1=qcos)

                def qk_chunk(mc):
                    ps_qk = psA.tile([128, NT], F32, tag="mm1",
                                     name=f"ps_qk_{it}_{mc}")
                    if QK8:
                        for p2 in range(KC // 2):
                            for nh in range(2):
                                ncol = slice(nh * 256, (nh + 1) * 256)
                                nc.tensor.matmul(
                                    ps_qk[:, ncol],
                                    wqk[:, 2 * p2 : 2 * p2 + 2,
                                        mc * 128 : (mc + 1) * 128],
                                    x_s[:, 2 * p2 : 2 * p2 + 2, ncol],
                                    start=(p2 == 0),
                                    stop=False,
                                    perf_mode=DR,
                                    skip_group_check=True,
                                )
                    else:
                        for kc in range(KC):
                            nc.tensor.matmul(
                                ps_qk[:, :],
                                wqk[:, kc, mc * 128 : (mc + 1) * 128],
                                x_s[:, kc, :],
                                start=(kc == 0), stop=False,
                            )
                    for h2 in range(2):
                        nc.tensor.matmul(
                            ps_qk[:, h2 * 256 : (h2 + 1) * 256],
                            aqk[:, :, mc * 128 : (mc + 1) * 128],
                            nmr2[:, :, h2 * 256 : (h2 + 1) * 256],
                            start=False, stop=True,
                            perf_mode=DR,
                            skip_group_check=True,
                        )
                    qsin = tpool.tile([128, NT], BF16, tag="tsin", bufs=2,
                                      name=f"qsin_{it}_{mc}")
                    nc.vector.scalar_tensor_tensor(
                        out=qsin, in0=ps_qk[:, :], scalar=bqk[:, mc : mc + 1],
                        in1=bcast_win(sinb, WPT),
                        op0=mybir.AluOpType.add, op1=mybir.AluOpType.mult,
                    )
                    qcos = tpool.tile([128, NT], BF16, tag="tcos", bufs=2,
                                      name=f"qcos_{it}_{mc}")
                    nc.vector.scalar_tensor_tensor(
                        out=qcos, in0=ps_qk[:, :], scalar=bqk[:, mc : mc + 1],
                        in1=bcast_win(cosb, WPT),
                        op0=mybir.AluOpType.add, op1=mybir.AluOpType.mult,
                    )
                    pend.append((mc, qsin, qcos))
                    if len(pend) > ROPE_LAG:
                        rope_tail(*pend.pop(0))

                def vt_group(g):
                    sub, nh = g // 2, g % 2
                    ncol = slice(nh * 512, (nh + 1) * 512)
                    ps_vt = psA.tile([128, NT], F32, tag="vtp", bufs=2,
                                     name=f"ps_vt_{it}_{sub}_{nh}")
                    if VT8:
                        for p2 in range(KC // 2):
                            for qh in range(2):
                                qcol = slice(nh * 512 + qh * 256,
                                             nh * 512 + (qh + 1) * 256)
                                nc.tensor.matmul(
                                    ps_vt[:, qh * 256 : (qh + 1) * 256],
                                    x_sv[:, 2 * p2 : 2 * p2 + 2,
                                         sub * 128 : (sub + 1) * 128],
                                    wv[:, 2 * p2 : 2 * p2 + 2, qcol],
                                    start=(p2 == 0),
                                    stop=False,
                                    perf_mode=DR,
                                    skip_group_check=True,
                                )
                    else:
                        for kc in range(KC):
                            nc.tensor.matmul(
                                ps_vt[:, :],
                                x_sv[:, kc, sub * 128 : (sub + 1) * 128],
                                wv[:, kc, ncol],
                                start=(kc == 0), stop=False,
                            )
                    for h2 in range(2):
                        nc.tensor.matmul(
                            ps_vt[:, h2 * 256 : (h2 + 1) * 256],
                            nmr2[:, :, sub * 128 : (sub + 1) * 128],
                            avbv[:, :, nh * 512 + h2 * 256 : nh * 512 + (h2 + 1) * 256],
                            start=False, stop=True,
                            perf_mode=DR,
                            skip_group_check=True,
                        )
                    if g % 2 == 0:
                        nc.scalar.activation(
                            out=vt[:, sub, ncol],
                            in_=ps_vt[:, :],
                            func=mybir.ActivationFunctionType.Identity,
                            bias=zero128[:, :], scale=s_vt,
                        )
                    else:
                        nc.vector.tensor_scalar(
                            out=vt[:, sub, ncol],
                            in0=ps_vt[:, :], scalar1=s_vt, scalar2=None,
                            op0=mybir.AluOpType.mult,
                        )

                for mc in range(16):
                    qk_chunk(mc)
                    if mc >= 8:
                        vt_group(mc - 8)
                while pend:
                    rope_tail(*pend.pop(0))
                st["qk"] = qk
                st["vt"] = vt

            def attn_pass1(st):
                it, qk = st["it"], st["qk"]
                _mark(nc, f'attn_{it}')
                expt = apool.tile([128, WPT, 2, 8 * WIN], BF16, tag="expt",
                                  bufs=1, name=f"expt_{it}")
                ps_d = psA.tile([HEADS, WPT * WIN], F32, tag="dd", bufs=1,
                                name=f"ps_d_{it}")

                def ecol(hh):
                    return (hh % 2) * 512 + (hh // 2) * WIN

                def scores(wl):
                    wslc = slice(wl * WIN, (wl + 1) * WIN)
                    # parity-split so matmuls with different contraction
                    # row-groups (operand base partition 0 vs 64) never share
                    # a PSUM bank (HW faults otherwise).
                    for hg in range(2):
                        for par in range(2):
                            ps_sc = psA.tile([128, 4 * WIN], F32, tag="mm1",
                                             name=f"ps_sc_{it}_{wl}_{hg}_{par}")
                            po = par * 64
                            for j in range(4):
                                h = hg * 8 + 2 * j + par
                                qh = qk[po : po + 64, h // 2, wslc]
                                kh = qk[po : po + 64, 8 + h // 2, wslc]
                                nc.tensor.matmul(
                                    ps_sc[:, j * WIN : (j + 1) * WIN],
                                    kh, qh, start=True, stop=True,
                                )
                            nc.scalar.activation(
                                out=expt[:, wl, hg, par * 512 : (par + 1) * 512],
                                in_=ps_sc[:, :],
                                func=mybir.ActivationFunctionType.Exp,
                                bias=zero128[:, :], scale=0.125,
                            )

                def denom(wl):
                    for hg in range(2):
                        for hh in range(8):
                            h = hg * 8 + hh
                            nc.tensor.matmul(
                                ps_d[:, wl * WIN : (wl + 1) * WIN],
                                e16[:, h, :],
                                expt[:, wl, hg, ecol(hh) : ecol(hh) + WIN],
                                start=(h == 0), stop=(h == HEADS - 1),
                            )

                # all scores first (releases the single-buffered qk tile
                # early for the next tile's rope writes), then denominators
                # (their expt inputs are ready by then)
                for wl in range(WPT):
                    scores(wl)
                for wl in range(WPT):
                    denom(wl)

                rd = spool.tile([HEADS, WPT * WIN], F32, tag="rd", bufs=1,
                                name=f"rd_{it}")
                nc.vector.reciprocal_approx_fast(out=rd, in_=ps_d[:, :])
                rd_bf = spool.tile([HEADS, WPT * WIN], BF16, tag="rdb", bufs=1,
                                   name=f"rd_bf_{it}")
                nc.scalar.mul(out=rd_bf, in_=rd,
                              mul=SW_X if OUT8 else 1.0)
                # head-major flat bounce: head h's 4-window row at offset
                # h*512; partition half ph reads heads 2c+ph (stride 1024)
                bcw = apool.tile([128, KC, WPT, WIN], BF16, tag="bc",
                                 bufs=1, name=f"bcw_{it}")
                rd_dram = dpool.tile([HEADS, WPT * WIN], BF16, tag="rd_dram",
                                     name=f"rd_dram_{it}")
                nc.sync.dma_start(out=rd_dram, in_=rd_bf)
                for ph in range(2):
                    srcap = bass.AP(
                        tensor=rd_dram.tensor,
                        offset=rd_dram.offset + ph * (WPT * WIN),
                        ap=[[0, 64], [2 * WPT * WIN, KC], [1, WPT * WIN]],
                    )
                    nc.sync.dma_start(
                        out=bcw[ph * 64 : (ph + 1) * 64], in_=srcap)
                st["expt"] = expt
                st["bcw"] = bcw

            def attn_pass2(st):
                it, vt, expt, bcw = st["it"], st["vt"], st["expt"], st["bcw"]
                attn_t = apool.tile([128, KC, NT], OUTD, tag="attn", bufs=1,
                                    name=f"attn_t_{it}")

                def ecol2(hh):
                    return (hh % 2) * 512 + (hh // 2) * WIN

                for wl in range(WPT):
                    wslc = slice(wl * WIN, (wl + 1) * WIN)
                    for hg in range(2):
                        ps_at = psA.tile([128, 4 * WIN], F32, tag="mm1",
                                         name=f"ps_at_{it}_{wl}_{hg}")
                        for hh in range(8):
                            h = hg * 8 + hh
                            po = (h % 2) * 64
                            c = (h // 2) % 4
                            nc.tensor.matmul(
                                ps_at[po : po + 64, c * WIN : (c + 1) * WIN],
                                vt[:, wl, h * 64 : (h + 1) * 64],
                                expt[:, wl, hg, ecol2(hh) : ecol2(hh) + WIN],
                                start=True, stop=True,
                                tile_position=(0, po),
                            )
                        nc.vector.tensor_mul(
                            out=attn_t[:, 4 * hg : 4 * hg + 4, wslc],
                            in0=ps_at.rearrange("p (c i) -> p c i", c=4),
                            in1=bcw[:, 4 * hg : 4 * hg + 4, wl, :],
                        )
                st["attn_t"] = attn_t

            def outproj_phase(st):
                it, tb, attn_t, xb = st["it"], st["tb"], st["attn_t"], st["xb"]
                _mark(nc, f'outproj_{it}')
                s_out = 1.0 / SW if OUT8 else 1.0
                for mc in range(KC):
                    ps_y = psA.tile([128, NT], F32, tag="mm1",
                                    name=f"ps_y_{it}_{mc}")
                    if OUT8:
                        for p2 in range(KC // 2):
                            for nh in range(2):
                                ncol = slice(nh * 256, (nh + 1) * 256)
                                nc.tensor.matmul(
                                    ps_y[:, ncol],
                                    wo[:, 2 * p2 : 2 * p2 + 2,
                                       mc * 128 : (mc + 1) * 128],
                                    attn_t[:, 2 * p2 : 2 * p2 + 2, ncol],
                                    start=(p2 == 0),
                                    stop=(p2 == KC // 2 - 1),
                                    perf_mode=DR,
                                    skip_group_check=True,
                                )
                    else:
                        for kc in range(KC):
                            nc.tensor.matmul(
                                ps_y[:, :],
                                wo[:, kc, mc * 128 : (mc + 1) * 128],
                                attn_t[:, kc, :],
                                start=(kc == 0), stop=(kc == KC - 1),
                            )
                    y = ypool.tile([128, NT], BF16, tag="y", bufs=3,
                                   name=f"y_{it}_{mc}")
                    nc.vector.scalar_tensor_tensor(
                        out=y, in0=ps_y[:, :], scalar=s_out, in1=xb[:, mc, :],
                        op0=mybir.AluOpType.mult, op1=mybir.AluOpType.add,
                    )
                    nc.sync.dma_start(
                        out=out_r[:, mc, tb : tb + NT], in_=y,
                    )

            # software pipeline: tile i+1's prologue+qkrope+vt are emitted
            # between tile i's attention and outproj, so PE's in-order stream
            # has GEMM work to chew on while attention waits on the softmax
            # denominator round-trip
            its = [t for _ in range(reps) for t in range(NTILES)]
            loads = {0: prologue_load(its[0])}
            if len(its) > 1:
                loads[1] = prologue_load(its[1])
            states = {0: prologue(loads.pop(0))}
            qkvt_phase(states[0])
            for idx, it in enumerate(its):
                st = states.pop(idx)
                if idx + 2 < len(its):
                    loads[idx + 2] = prologue_load(its[idx + 2])
                if idx + 1 < len(its):
                    states[idx + 1] = prologue(loads.pop(idx + 1))
                attn_pass1(st)
                attn_pass2(st)
                if idx + 1 < len(its):
                    qkvt_phase(states[idx + 1])
                outproj_phase(st)
    nc.finalize()
    return nc


def _host_prep(x, ln_w, ln_b, w_qkv, w_out):
    """Shared (non-x) device inputs, host-precomputed."""
    bf = ml_dtypes.bfloat16
    f8 = ml_dtypes.float8_e4m3fn
    wqkv_s = (w_qkv * ln_w[None, :]).astype(np.float32)  # (3C, C) scaled
    wT = np.ascontiguousarray(wqkv_s.T)  # (C, 3C)
    b_qkv = (w_qkv @ ln_b).astype(np.float32)  # (3C,)
    a_qkv = wqkv_s.sum(axis=1).astype(np.float32)  # (3C,)

    QK8, VT8, OUT8 = CFG["fp8_qk"], CFG["fp8_vt"], CFG["fp8_out"]
    ins = {}
    ins["wqk"] = (
        np.ascontiguousarray(wT[:, : 2 * DIM]) * (SW_W if QK8 else 1.0)
    ).astype(f8 if QK8 else bf)
    ins["wv"] = (
        np.ascontiguousarray(wT[:, 2 * DIM :]) * (SW_W if VT8 else 1.0)
    ).astype(f8 if VT8 else bf)
    ins["wo"] = (
        np.ascontiguousarray(w_out.T) * (SW_W if OUT8 else 1.0)
    ).astype(f8 if OUT8 else bf)
    aqk2 = np.zeros((1, 2, 2 * DIM), np.float32)
    aqk2[0, 0, :] = a_qkv[: 2 * DIM]
    ins["aqk"] = aqk2.astype(f8)
    av2 = np.zeros((1, 2, DIM), np.float32)
    av2[0, 0, :] = a_qkv[2 * DIM :]
    ins["avbv"] = av2.astype(f8)
    # q,k bias as (partition, chunk), divided by s_qk (the rope tables carry
    # s_qk, so (ps + b/s)*(t*s) = (ps*s + b)*t)
    ins["bqk"] = np.ascontiguousarray(
        b_qkv[: 2 * DIM].reshape(16, 128).T / (1.0 / (SW if QK8 else SW_X))
    ).astype(np.float32)

    s_qk = 1.0 / (SW if QK8 else SW_X)
    inv_freq = 1.0 / 10000 ** (np.arange(0, DHEAD, 2, dtype=np.float32) / DHEAD)
    pos = np.arange(WIN, dtype=np.float32)
    freqs = np.concatenate([np.outer(pos, inv_freq)] * 2, axis=-1)  # (WIN, 64)
    cos_t = np.cos(freqs).T.astype(np.float32) * s_qk  # (64, WIN)
    sin_t = np.sin(freqs).T.astype(np.float32) * s_qk
    ins["cosb"] = np.tile(cos_t, (2, 1)).astype(bf)
    ins["sinb"] = np.tile(sin_t, (2, 1)).astype(bf)

    S = np.zeros((DHEAD, DHEAD), np.float32)
    S[: DHEAD // 2, DHEAD // 2 :] = -np.eye(DHEAD // 2)
    S[DHEAD // 2 :, : DHEAD // 2] = np.eye(DHEAD // 2)
    ST = S.T
    st128 = np.zeros((128, 128), np.float32)
    st128[:64, :64] = ST
    st128[64:, 64:] = ST
    ins["st128"] = st128.astype(bf)

    e = np.zeros((128, HEADS, HEADS), np.float32)
    for h in range(HEADS):
        e[:, h, h] = 1.0
    ins["e16"] = e.astype(bf)


    ins["ones_col"] = np.ones((128, 1), np.float32).astype(bf)
    ins["ones_row"] = np.ones((1, 128), np.float32).astype(bf)
    return ins


def kernel(x, ln_w, ln_b, w_qkv, w_out, _want_trace=False):
    x = np.asarray(x, dtype=np.float32)
    shared = _host_prep(
        np.asarray(x, np.float32),
        np.asarray(ln_w, np.float32),
        np.asarray(ln_b, np.float32),
        np.asarray(w_qkv, np.float32),
        np.asarray(w_out, np.float32),
    )

    if "nc" not in _CACHED:
        _CACHED["nc"] = _build_bass()
    nc = _CACHED["nc"]

    x_bf = x.astype(ml_dtypes.bfloat16)
    in_maps = []
    for core in range(NCORE):
        b, half = core // 2, core % 2
        xs = np.ascontiguousarray(x_bf[b, :, half * NTOK : (half + 1) * NTOK])
        m = dict(shared)
        m["x"] = xs
        in_maps.append(m)

    res = run_bass_kernel_spmd(
        nc, in_maps, core_ids=list(range(NCORE)), trace=_want_trace
    )
    out = np.empty((B, DIM, T), np.float32)
    for core in range(NCORE):
        b, half = core // 2, core % 2
        out[b, :, half * NTOK : (half + 1) * NTOK] = (
            res.results[core]["out"].astype(np.float32)
        )
    if _want_trace:
        _CACHED["last_trace"] = res
    return out

